# revision 39
# baseline (speedup 1.0000x reference)
"""Trainium2 Bass kernel for sliding-window GQA attention block.

Module: q/k/v projections -> per-head RMSNorm(q,k) -> RoPE -> sliding-window
causal attention (window=1024, GQA 16 q heads / 4 kv heads) -> out projection.

Sharding (8 cores, tensor parallel over heads):
  core c owns q heads {2c, 2c+1} and kv head c//2.
  Each core computes attention for its 2 heads and a partial out-projection
  (contraction over its 256 head-features); the host sums the 8 partials.

Schedule (v2):
  - Phase A: QKV projection, k-outer so the xt stream (8.4 MB) hides behind
    the PE. Pass 1 accumulates k+v into 8 PSUM banks over all 16 d-slices;
    V is PE-transposed into natural [S, hd] blocks; pass 2 does q0+q1.
  - Phase B: per-512-tile RMSNorm + RoPE in the transposed layout, fed to
    attention tile-by-tile. The 1/rms row is partition-broadcast on GpSimd
    (no DRAM bounce).
  - Phase C: attention per (head, q-tile). Scores per kv-block pair into
    PSUM, exp on ACT, mask on DVE; denominator (ones-matmul) and PV are
    issued per block with column ranges trimmed to the sliding window.
    Softmax reciprocal is broadcast across partitions on GpSimd. q1's
    norm/rope runs on DVE/ACT while h=0 attention owns the PE.
  - Phase D: partial out-projection; PSUM halves evicted alternately on
    ACT/DVE into bf16 staging tiles, DMA'd out per [128,1024] tile.

Precision: matmul-fed tensors are bf16 (KERNEL_DTYPE=f32 for full fp32);
softmax denominator chain is fp32; the partial output is written bf16
(KERNEL_OUT=f32 for fp32) and summed on the host in fp32.
"""

import os
import sys

for _p in ("/opt/trn_rl_repo", "/root/.axon_site/_ro/trn_rl_repo"):
    if _p not in sys.path:
        sys.path.insert(0, _p)

import numpy as np

N_HEADS = 16
N_KV_HEADS = 4
HEAD_DIM = 128
D_MODEL = 2048
WINDOW = 1024
THETA = 10000.0
EPS = 1e-6
S = 2048
B = 1
N_CORES = 8
KD = D_MODEL // 128          # 16 contraction tiles over d_model
NQT = S // 512               # 4 q tiles of 512
SCALE = HEAD_DIM ** -0.5

DTYPE_MODE = os.environ.get("KERNEL_DTYPE", "bf16")
OUT_MODE = os.environ.get("KERNEL_OUT", "bf16")

# mask tile ids by delta0 = qstart - kvstart
_MASK_D0 = [0, -128, -256, -384, 640, 768, 896, 1024]
_MASK_IDX = {d0: i for i, d0 in enumerate(_MASK_D0)}


def _kv_blocks(t):
    """kv 128-blocks needed by q tile t (512 queries); the diagonal block
    (b=4t, full q-width) leads so it opens the PSUM accumulation groups."""
    blocks = list(range(max(0, 4 * t - 8), 4 * t + 4))
    blocks.remove(4 * t)
    return [4 * t] + blocks


def _tile_pairs(t):
    """kv-block pairs for q tile t, ordered so that (a) the diagonal block
    (full q-width) opens the PSUM accumulation groups, and (b) each pair
    packs a suffix-trimmed block (slot A) with a prefix/full block (slot B)
    so the pair's exp covers one contiguous, trimmed column range."""
    def d0(b):
        return 512 * t - 128 * b
    blocks = list(range(max(0, 4 * t - 8), 4 * t + 4))
    diag = 4 * t
    prefixes = sorted((b for b in blocks if d0(b) >= 640), key=lambda b: -d0(b))
    suffixes = sorted((b for b in blocks if b != diag and d0(b) <= 0),
                      key=lambda b: d0(b))
    fulls = [b for b in blocks if 128 <= d0(b) <= 512]
    rest = prefixes + fulls
    pairs = [(diag, rest.pop(0)) if rest else (diag, suffixes.pop(0))]
    while suffixes:
        sfx = suffixes.pop(0)
        pairs.append((sfx, rest.pop(0) if rest else suffixes.pop()))
    while rest:
        pairs.append((rest.pop(0), rest.pop(0)))
    return pairs


def _mask_idx(t, b):
    d0 = 512 * t - 128 * b
    return _MASK_IDX.get(d0) if (d0 <= 0 or d0 > 512) else None


def _trim(t, b):
    """valid q-column range [j0, j1) of block b within q tile t."""
    d0 = 512 * t - 128 * b
    if d0 <= 0:
        return -d0, 512
    return 0, min(512, 1152 - d0)


def _build_program():
    import concourse.bass as bass  # noqa: F401
    import concourse.bacc as bacc
    import concourse.tile as tile
    from concourse import mybir
    from concourse.masks import make_identity

    f32 = mybir.dt.float32
    sd = mybir.dt.bfloat16 if DTYPE_MODE == "bf16" else f32
    od = mybir.dt.bfloat16 if OUT_MODE == "bf16" else f32
    AF = mybir.ActivationFunctionType

    nc = bacc.Bacc("TRN2", target_bir_lowering=False, debug=False)

    # host-pretiled: xt_t[p][k][s] = x[s, 128k+p]; wcat_t[p] holds the
    # [kt, m] weight tiles for partition p; similarly wot_t / masks.
    xt_d = nc.dram_tensor("xt", [128, KD, S], sd, kind="ExternalInput").ap()
    wcat_d = nc.dram_tensor("wcat", [128, KD, 512], sd, kind="ExternalInput").ap()
    wot_d = nc.dram_tensor("wot", [128, 2, D_MODEL], sd, kind="ExternalInput").ap()
    cs2q_d = nc.dram_tensor("cs2q", [128, S], sd, kind="ExternalInput").ap()
    ss2q_d = nc.dram_tensor("ss2q", [128, S], sd, kind="ExternalInput").ap()
    cs2k_d = nc.dram_tensor("cs2k", [128, S], sd, kind="ExternalInput").ap()
    ss2k_d = nc.dram_tensor("ss2k", [128, S], sd, kind="ExternalInput").ap()
    masks_d = nc.dram_tensor("masks", [128, 8, 512], sd, kind="ExternalInput").ap()
    out_d = nc.dram_tensor("out", [S, D_MODEL], od, kind="ExternalOutput").ap()

    with tile.TileContext(nc) as tc:
        with tc.tile_pool(name="persist", bufs=1) as persist:
            # q0, q1, k transposed [128 hd, S]; start as pre-rope, finalized in place
            qkv = [persist.tile([128, S], sd, tag=f"qkv{m}", name=f"qkv{m}") for m in range(3)]
            vnat = persist.tile([128, KD, HEAD_DIM], sd, tag="vnat")
            oT = [persist.tile([128, S], sd, tag=f"oT{h}", name=f"oT{h}") for h in range(2)]
            ones_col = persist.tile([128, 1], sd, tag="ones_col")
            nc.vector.memset(ones_col, 1.0)
            ident = persist.tile([128, 128], sd, tag="ident")
            make_identity(nc, ident)
            mean_col = persist.tile([128, 1], sd, tag="mean_col")
            nc.vector.memset(mean_col, 1.0 / HEAD_DIM)
            ones_row = persist.tile([1, 128], f32, tag="ones_row")
            nc.vector.memset(ones_row, 1.0)
            sclP = persist.tile([128, 1], f32, tag="sclP")
            nc.vector.memset(sclP, SCALE)
            eps1 = persist.tile([1, 1], f32, tag="eps1")
            nc.vector.memset(eps1, EPS)
            eps128 = persist.tile([128, 1], f32, tag="eps128")
            nc.vector.memset(eps128, EPS)

            # SBUF pools that live through phases A-C
            sb_pools = (
                tc.tile_pool(name="rotp", bufs=1),
                tc.tile_pool(name="avt", bufs=1),
                tc.tile_pool(name="btmp", bufs=4),
                tc.tile_pool(name="bsm", bufs=3),
            )
            rotp, avt, btp, bsm = (p.__enter__() for p in sb_pools)
            rot = [rotp.tile([128, S], sd, tag=f"rot{m}", name=f"rot{m}") for m in range(3)]

            # input pools, released after phase A
            in_pools = (
                tc.tile_pool(name="aw", bufs=1),
                tc.tile_pool(name="ax", bufs=1),
            )
            awp, axp = (p.__enter__() for p in in_pools)
            wcat_sb = awp.tile([128, KD, 512], sd)
            xt_sb = axp.tile([128, KD, S], sd)
            # chunked input streams: a small weight chunk first so the
            # k-outer loop starts as early as possible (sync/HWDGE only -
            # gpsimd SWDGE-issued DMAs throttle the stream)
            nc.sync.dma_start(out=wcat_sb[:, 0:2, :], in_=wcat_d[:, 0:2, :])
            nc.sync.dma_start(out=xt_sb[:, 0:1, :], in_=xt_d[:, 0:1, :])
            nc.sync.dma_start(out=wcat_sb[:, 2:4, :], in_=wcat_d[:, 2:4, :])
            nc.sync.dma_start(out=xt_sb[:, 1:2, :], in_=xt_d[:, 1:2, :])
            for i in range(1, KD // 2):
                nc.sync.dma_start(out=xt_sb[:, 2 * i:2 * i + 2, :], in_=xt_d[:, 2 * i:2 * i + 2, :])
                if i == 1:
                    nc.sync.dma_start(out=wcat_sb[:, 4:8, :], in_=wcat_d[:, 4:8, :])
                if i == 2:
                    nc.sync.dma_start(out=wcat_sb[:, 8:16, :], in_=wcat_d[:, 8:16, :])

            # warm the ACT function tables while the inputs stream
            dummy = persist.tile([1, 1], f32, tag="dummy")
            nc.scalar.activation(dummy, eps1, AF.Square)
            nc.scalar.activation(dummy, eps1, AF.Sqrt)
            nc.scalar.activation(dummy, eps1, AF.Exp)

            # constants for phases B/C/D (after the critical-path loads)
            cs_sb = {}
            for nm, dd in (("cs2k", cs2k_d), ("ss2k", ss2k_d),
                           ("cs2q", cs2q_d), ("ss2q", ss2q_d)):
                t_ = persist.tile([128, S], sd, tag=nm, name=nm)
                nc.sync.dma_start(out=t_, in_=dd)
                cs_sb[nm] = t_
            masks_sb = persist.tile([128, 8, 512], sd, tag="masks_sb")
            nc.sync.dma_start(out=masks_sb, in_=masks_d)
            wot_sb = persist.tile([128, 2, D_MODEL], sd, tag="wot_sb")
            nc.sync.dma_start(out=wot_sb, in_=wot_d)

            # ---------------- Phase A: projections, k-outer ----------------
            # ---------------- Phase B: per-tile rms norm + rope -------------
            from concourse import bass_isa

            def norm_tile(m, t, pspool, pstag):
                """rms norm + rope on qkv[m][:, 512t:512(t+1)], in place"""
                ts_ = slice(t * 512, (t + 1) * 512)
                pre = qkv[m]
                rt = rot[m]
                csx = cs_sb["cs2q" if m < 2 else "cs2k"]
                ssx = cs_sb["ss2q" if m < 2 else "ss2k"]
                sq = btp.tile([128, 512], sd, tag="sq")
                nc.scalar.activation(sq, pre[:, ts_], AF.Square)
                ps = pspool.tile([1, 512], f32, tag=pstag)
                nc.tensor.matmul(ps, lhsT=mean_col, rhs=sq)
                ssc = bsm.tile([1, 512], f32, tag="ssc")
                nc.scalar.activation(ssc, ps, AF.Sqrt, bias=eps1)
                ssr = bsm.tile([1, 512], f32, tag="ssr")
                nc.vector.reciprocal_approx_fast(ssr, ssc)
                rnb = btp.tile([128, 512], f32, tag="rnb")
                nc.gpsimd.partition_broadcast(rnb, ssr, channels=128)
                # rope: final = (pre*cs + rot*ss) * rn, in place (2-operand
                # DVE ops run ~2x faster than 3-operand ones)
                nc.vector.tensor_mul(pre[:, ts_], pre[:, ts_], csx[:, ts_])
                nc.vector.tensor_mul(rt[:, ts_], rt[:, ts_], ssx[:, ts_])
                nc.vector.tensor_add(pre[:, ts_], pre[:, ts_], rt[:, ts_])
                nc.vector.tensor_mul(pre[:, ts_], pre[:, ts_], rnb)

            # two independent 4-bank pools: pass 2 runs single-tensor
            # passes in apA while the norm chains pipeline in apB's banks
            apA_ctx = tc.tile_pool(name="apA", bufs=1, space="PSUM")
            apA = apA_ctx.__enter__()
            apB_ctx = tc.tile_pool(name="apB", bufs=1, space="PSUM")
            apB = apB_ctx.__enter__()

            def proj_mms(psA, m, k):
                for n in range(4):
                    nc.tensor.matmul(
                        psA[:, n * 512:(n + 1) * 512],
                        lhsT=wcat_sb[:, k, m * 128:(m + 1) * 128],
                        rhs=xt_sb[:, k, n * 512:(n + 1) * 512],
                        start=(k == 0),
                        stop=(k == KD - 1),
                    )

            def evict(dst, src, m=None):
                """psum -> sbuf eviction, split across ACT/DVE"""
                nc.scalar.copy(dst[:, 0:1024], src[:, 0:1024])
                nc.vector.tensor_copy(dst[:, 1024:2048], src[:, 1024:2048])
                if m is not None:
                    nc.sync.dma_start(out=rot[m][64:128, :], in_=dst[0:64, :])
                    nc.sync.dma_start(out=rot[m][0:64, :], in_=dst[64:128, :])

            # pass 1: k + v interleaved k-outer (hides the xt stream)
            psA2 = apA.tile([128, S], f32, tag="psA", name="psA2")
            psA3 = apB.tile([128, S], f32, tag="psB", name="psA3")
            for k in range(KD):
                proj_mms(psA2, 2, k)
                proj_mms(psA3, 3, k)
            evict(qkv[2], psA2, 2)
            vtmp = avt.tile([128, S], sd, tag="vtmp")
            evict(vtmp, psA3)
            # transpose v into natural [S, hd] blocks (reuses the psA slots)
            for half in range(2):
                pool, tag = ((apA, "psA"), (apB, "psB"))[half]
                tp = pool.tile([128, 1024], sd, tag=tag, name=f"tp{half}")
                for j in range(8):
                    jj = half * 8 + j
                    nc.tensor.transpose(
                        tp[:, j * 128:(j + 1) * 128],
                        vtmp[:, jj * 128:(jj + 1) * 128], ident,
                    )
                nc.vector.tensor_copy(vnat[:, half * 8:(half + 1) * 8, :], tp)

            # free apB's banks for the norm ps tiles
            apB_ctx.__exit__(None, None, None)
            bps_ctx = tc.tile_pool(name="bps", bufs=4, space="PSUM")
            bps = bps_ctx.__enter__()

            # k's norm chains overlap the q0 pass
            for t in range(NQT):
                norm_tile(2, t, bps, "ssps")

            psA0 = apA.tile([128, S], f32, tag="psA", name="psA0")
            for k in range(KD):
                proj_mms(psA0, 0, k)
            evict(qkv[0], psA0, 0)
            # q0's norm chains overlap the q1 pass
            for t in range(NQT):
                norm_tile(0, t, bps, "ssps")

            psA1 = apA.tile([128, S], f32, tag="psA", name="psA1")
            for k in range(KD):
                proj_mms(psA1, 1, k)
            evict(qkv[1], psA1, 1)
            for t in range(NQT):
                norm_tile(1, t, bps, "ssps")

            for p in reversed(in_pools):
                p.__exit__(None, None, None)
            bps_ctx.__exit__(None, None, None)
            apA_ctx.__exit__(None, None, None)

            def norm_tile_gps(m, t):
                """psum-free rms norm + rope (partition sum on GpSimd) for
                overlap inside phase C"""
                ts_ = slice(t * 512, (t + 1) * 512)
                pre = qkv[m]
                rt = rot[m]
                csx = cs_sb["cs2q" if m < 2 else "cs2k"]
                ssx = cs_sb["ss2q" if m < 2 else "ss2k"]
                sq = btp.tile([128, 512], sd, tag="sq")
                nc.scalar.activation(sq, pre[:, ts_], AF.Square)
                parsum = btp.tile([128, 512], f32, tag="parsum")
                nc.gpsimd.partition_all_reduce(
                    parsum, sq, channels=128, reduce_op=bass_isa.ReduceOp.add)
                rsq = btp.tile([128, 512], f32, tag="rsq")
                nc.scalar.activation(rsq, parsum, AF.Sqrt,
                                     scale=1.0 / HEAD_DIM, bias=eps128)
                rnb = btp.tile([128, 512], f32, tag="rnb")
                nc.vector.reciprocal_approx_fast(rnb, rsq)
                nc.vector.tensor_mul(pre[:, ts_], pre[:, ts_], csx[:, ts_])
                nc.vector.tensor_mul(rt[:, ts_], rt[:, ts_], ssx[:, ts_])
                nc.vector.tensor_add(pre[:, ts_], pre[:, ts_], rt[:, ts_])
                nc.vector.tensor_mul(pre[:, ts_], pre[:, ts_], rnb)

            # ---------------- Phase C: windowed attention --------------------
            kT = qkv[2]
            with (
                tc.tile_pool(name="cpt", bufs=5) as cptp,
                tc.tile_pool(name="csm", bufs=3) as csmp,
                tc.tile_pool(name="csc", bufs=2, space="PSUM") as cscp,
                tc.tile_pool(name="cacc", bufs=2, space="PSUM") as caccp,
                tc.tile_pool(name="cden", bufs=2, space="PSUM") as cdenp,
            ):
                for h in range(2):
                    qT = qkv[h]
                    for t in range(NQT):
                        ts_ = slice(t * 512, (t + 1) * 512)
                        pairs = _tile_pairs(t)
                        first_b = pairs[0][0]
                        last_b = pairs[-1][-1]
                        psO = caccp.tile([128, 512], f32, tag="psO")
                        psD = cdenp.tile([1, 512], f32, tag="psD")
                        for pr in pairs:
                            ps = cscp.tile([128, 1024], f32, tag="sc")
                            pt = cptp.tile([128, 1024], sd, tag="pt")
                            trims = [_trim(t, b) for b in pr]
                            for jj, b in enumerate(pr):
                                j0, j1 = trims[jj]
                                nc.tensor.matmul(
                                    ps[:, jj * 512 + j0:jj * 512 + j1],
                                    lhsT=kT[:, b * 128:(b + 1) * 128],
                                    rhs=qT[:, t * 512 + j0:t * 512 + j1],
                                )
                            if trims[1][0] == 0:
                                e0, e1 = trims[0][0], 512 + trims[1][1]
                                nc.scalar.activation(pt[:, e0:e1], ps[:, e0:e1],
                                                     AF.Exp, scale=sclP)
                            else:
                                # slots not contiguous (t=0): exp each slot
                                for jj in range(2):
                                    j0, j1 = trims[jj]
                                    sl = slice(jj * 512 + j0, jj * 512 + j1)
                                    nc.scalar.activation(pt[:, sl], ps[:, sl],
                                                         AF.Exp, scale=sclP)
                            for jj, b in enumerate(pr):
                                mi = _mask_idx(t, b)
                                if mi is not None:
                                    j0, j1 = trims[jj]
                                    nc.vector.tensor_mul(
                                        pt[:, jj * 512 + j0:jj * 512 + j1],
                                        pt[:, jj * 512 + j0:jj * 512 + j1],
                                        masks_sb[:, mi, j0:j1],
                                    )
                            for jj, b in enumerate(pr):
                                j0, j1 = trims[jj]
                                pts = pt[:, jj * 512 + j0:jj * 512 + j1]
                                nc.tensor.matmul(
                                    psD[0:1, j0:j1], lhsT=ones_col, rhs=pts,
                                    start=(b == first_b), stop=(b == last_b),
                                )
                                nc.tensor.matmul(
                                    psO[:, j0:j1], lhsT=vnat[:, b, :], rhs=pts,
                                    start=(b == first_b), stop=(b == last_b),
                                )
                        # denominator -> reciprocal -> broadcast -> normalize
                        den_sb = csmp.tile([1, 512], f32, tag="den")
                        nc.vector.reciprocal_approx_fast(den_sb, psD)
                        recb = csmp.tile([128, 512], f32, tag="recb")
                        nc.gpsimd.partition_broadcast(recb, den_sb, channels=128)
                        nc.vector.tensor_mul(oT[h][:, ts_], psO, recb)


            for p in reversed(sb_pools):
                p.__exit__(None, None, None)

            # ---------------- Phase D: partial out projection ----------------
            with (
                tc.tile_pool(name="dout", bufs=8) as doutp,
                tc.tile_pool(name="dps", bufs=4, space="PSUM") as dpsp,
            ):
                for tq in range(16):
                    for ph in range(2):
                        po = dpsp.tile([128, 1024], f32, tag="po", name=f"po{tq}_{ph}")
                        for hh in range(2):
                            for d2 in range(2):
                                dn = 2 * ph + d2
                                nc.tensor.matmul(
                                    po[:, d2 * 512:(d2 + 1) * 512],
                                    lhsT=oT[hh][:, tq * 128:(tq + 1) * 128],
                                    rhs=wot_sb[:, hh, dn * 512:(dn + 1) * 512],
                                    start=(hh == 0), stop=(hh == 1),
                                )
                        ob = doutp.tile([128, 1024], od, tag="ob")
                        if ph == 0:
                            nc.scalar.copy(ob, po)
                        else:
                            nc.vector.tensor_copy(ob, po)
                        # alternate issue engines so out-DMA descriptor
                        # generation is not serialized on the sync queue
                        eng = (nc.sync, nc.gpsimd)[(2 * tq + ph) % 2]
                        eng.dma_start(
                            out=out_d[tq * 128:(tq + 1) * 128, ph * 1024:(ph + 1) * 1024],
                            in_=ob,
                        )

    nc.compile()
    return nc


def _host_shards(x, wq, wk, wv, wo, q_norm_w, k_norm_w):
    if DTYPE_MODE == "bf16":
        import ml_dtypes
        sdt = ml_dtypes.bfloat16
    else:
        sdt = np.float32

    x2 = np.asarray(x, np.float32).reshape(S, D_MODEL)
    # [128, KD, S]: xt[p, k, s] = x[s, 128k+p]
    xt = np.ascontiguousarray(x2.T.reshape(KD, 128, S).transpose(1, 0, 2)).astype(sdt)

    inv = 1.0 / (THETA ** (np.arange(0, HEAD_DIM, 2, dtype=np.float64) / HEAD_DIM))
    ang = np.arange(S, dtype=np.float64)[:, None] * inv[None, :]  # [S, 64]
    cos = np.cos(ang).T.astype(np.float32)  # [64, S]
    sin = np.sin(ang).T.astype(np.float32)

    def cs_ss(w):
        w = np.asarray(w, np.float32)
        wrot = np.concatenate([w[64:], w[:64]])
        cs2 = np.concatenate([cos, cos], 0) * w[:, None]
        ss2 = np.concatenate([-sin, sin], 0) * wrot[:, None]
        return np.ascontiguousarray(cs2).astype(sdt), np.ascontiguousarray(ss2).astype(sdt)

    cs2q, ss2q = cs_ss(q_norm_w)
    cs2k, ss2k = cs_ss(k_norm_w)

    masks = np.zeros((8, 128, 512), np.float32)
    ii = np.arange(128)[:, None]
    jj = np.arange(512)[None, :]
    for mi, d0 in enumerate(_MASK_D0):
        d = d0 + jj - ii
        masks[mi] = ((d >= 0) & (d < WINDOW)).astype(np.float32)
    masks_t = np.ascontiguousarray(masks.transpose(1, 0, 2)).astype(sdt)  # [128, 8, 512]

    wq = np.asarray(wq, np.float32)
    wk = np.asarray(wk, np.float32)
    wv = np.asarray(wv, np.float32)
    wo = np.asarray(wo, np.float32)

    in_maps = []
    for c in range(N_CORES):
        g = c // 2
        wcat = np.concatenate(
            [wq[256 * c:256 * (c + 1)], wk[128 * g:128 * (g + 1)], wv[128 * g:128 * (g + 1)]],
            axis=0,
        )  # [512, D]
        wcat_t = np.ascontiguousarray(
            wcat.T.reshape(KD, 128, 512).transpose(1, 0, 2)
        ).astype(sdt)  # [128, KD, 512]
        wot_t = np.ascontiguousarray(
            wo[:, 256 * c:256 * (c + 1)].T.reshape(2, 128, D_MODEL).transpose(1, 0, 2)
        ).astype(sdt)  # [128, 2, D]
        in_maps.append({
            "xt": xt,
            "wcat": wcat_t,
            "wot": wot_t,
            "cs2q": cs2q, "ss2q": ss2q, "cs2k": cs2k, "ss2k": ss2k,
            "masks": masks_t,
        })
    return in_maps


_NC_CACHE = None


def run_with_results(x, wq, wk, wv, wo, q_norm_w, k_norm_w, trace=False):
    global _NC_CACHE
    from concourse.bass_utils import run_bass_kernel_spmd

    if _NC_CACHE is None:
        _NC_CACHE = _build_program()
    nc = _NC_CACHE
    in_maps = _host_shards(x, wq, wk, wv, wo, q_norm_w, k_norm_w)
    res = run_bass_kernel_spmd(nc, in_maps, list(range(N_CORES)), trace=trace)
    parts = np.stack([np.asarray(res.results[i]["out"], np.float32) for i in range(N_CORES)], axis=0)
    out = parts.sum(axis=0, dtype=np.float32).reshape(B, S, D_MODEL)
    return np.ascontiguousarray(out.astype(np.float32)), res


def kernel(x, wq, wk, wv, wo, q_norm_w, k_norm_w):
    out, _ = run_with_results(x, wq, wk, wv, wo, q_norm_w, k_norm_w, trace=False)
    return out
